# revision 13
# baseline (speedup 1.0000x reference)
"""Trainium2 Bass kernel for nn_DecompMultiTransform (RGCN basis-decomposition).

Reference computation:
    full_w = (w_comp @ weight).reshape(64, 256, 256)   # per-type weights
    out[n, :] = x[n, :] @ full_w[xtype[n]]             # N = 4096

Kernel formulation (avoids materializing the 16 MB full_w):
    onehot[t, n] = (xtype[n] == t)                     # [64, 512] per core
    cb_b[n]      = w_comp[xtype[n], b]  (via matmul with onehot)
    u_b[p, ihn]  = x^T * cb_b                          # scaled x halves
    outT[o, n]   = sum_{b,ih} weight-chunk^T @ u_b     # K=128 dense matmuls

Sharding: data-parallel over N across 8 cores (512 rows each); weight and
w_comp replicated. All math (type lookup, scaling, matmuls) runs on device;
host only does layout (transpose/pack) and dtype casts.

v4 changes vs v3 (45.1us):
  - the [64,512] xtype partition-broadcast DMA took 3.8us (stride-0
    descriptor replay) and serialized the whole sync queue behind it.
    Replaced with a 1KB [1,512] upload + K=1 ones-matmul broadcast on the
    PE; onehot compares the PSUM copy against an fp32 iota.
  - sync queue reordered: 3KB of criticals, then xT, wt0, then the cb
    bounce + 3 broadcast reads; bulk weights on the scalar queue.
  - 3 head bases on the PE-broadcast path (hides the bounce latency with
    zero PE gaps); 13 via the DRAM bounce.
"""

import sys

if "/opt/trn_rl_repo" not in sys.path:
    sys.path.insert(0, "/opt/trn_rl_repo")

import numpy as np
import ml_dtypes

import concourse.bass as bass
import concourse.mybir as mybir
import concourse.tile as tile
from concourse import bacc
from concourse.bass_utils import run_bass_kernel_spmd

P = 128
N_FULL = 4096
IN_DIM = 256
OUT_DIM = 256
NUM_B = 16
NUM_T = 64
N_CORES = 8
ROWS = N_FULL // N_CORES          # 512 rows per core
KT = NUM_B * (IN_DIM // P)        # 32 contraction tiles of 128
W_BATCHES = (2, 2, 4, 8)          # weight DMA batch sizes (bases)
PE_BASES = 5                      # head bases with PE-broadcast cb
CB_GROUPS = ((5, 9), (9, 13), (13, 16))  # DRAM-bounce broadcast groups
N_DUMMY = 3                       # PE p-state warmup matmuls

F32 = mybir.dt.float32
BF16 = mybir.dt.bfloat16
I32 = mybir.dt.int32
NPBF16 = ml_dtypes.bfloat16


def _build_program():
    nc = bacc.Bacc("TRN2", target_bir_lowering=False, debug=False)

    xT = nc.declare_dram_parameter("xT", [P, 2 * ROWS], BF16, isOutput=False)
    xtypeb = nc.declare_dram_parameter("xtypeb", [1, ROWS], BF16, isOutput=False)
    wcomp = nc.declare_dram_parameter("wcomp", [NUM_T, NUM_B], BF16, isOutput=False)
    # packed weight: wt[p, b, ih, o] = weight[b, (ih*128+p)*256 + o]
    wt = nc.declare_dram_parameter("wt", [P, NUM_B * 2 * OUT_DIM], BF16, isOutput=False)
    outT = nc.declare_dram_parameter("outT", [OUT_DIM, ROWS], F32, isOutput=True)

    cbdram = nc.dram_tensor("cb_bounce", [NUM_B, ROWS], BF16)
    wtv = wt.ap().rearrange("p (b r) -> p b r", b=NUM_B)

    with tile.TileContext(nc) as tc:
        with (
            tc.tile_pool(name="const", bufs=1) as constp,
            tc.tile_pool(name="wpool", bufs=1) as wpool,
            tc.tile_pool(name="cbp", bufs=2) as cbp,
            tc.tile_pool(name="up", bufs=4) as up,
            tc.tile_pool(name="outp", bufs=1) as outp,
            tc.tile_pool(name="psb", bufs=2, space="PSUM") as psb,
            tc.tile_pool(name="pso", bufs=1, space="PSUM") as pso,
        ):
            # ---- PE warmup scratch + broadcast helpers (no DMA deps) ----
            dlhs = constp.tile([P, P], BF16, name="dlhs")
            drhs = constp.tile([P, ROWS], BF16, name="drhs")
            ones = constp.tile([1, NUM_T], BF16, name="ones")
            nc.vector.memset(dlhs[:], 0)
            nc.vector.memset(drhs[:], 0)
            nc.vector.memset(ones[:], 1.0)
            dps = pso.tile([P, ROWS], F32, name="dps", space="PSUM")
            for _ in range(N_DUMMY):
                nc.tensor.matmul(
                    out=dps[:], lhsT=dlhs[:], rhs=drhs[:], start=True, stop=True
                )

            # ---- sync-queue DMAs in priority order ----
            xtype_sb = constp.tile([1, ROWS], BF16, name="xtype_sb")
            nc.sync.dma_start(out=xtype_sb[:], in_=xtypeb.ap()[:, :])

            wcomp_sb = constp.tile([NUM_T, NUM_B], BF16, name="wcomp_sb")
            nc.sync.dma_start(out=wcomp_sb[:], in_=wcomp.ap()[:, :])

            xtcat = constp.tile([P, 2 * ROWS], BF16, name="xtcat")
            nc.sync.dma_start(out=xtcat, in_=xT.ap()[:, :])

            wtbs = []
            b0 = 0
            for k, nb in enumerate(W_BATCHES):
                wtb = wpool.tile([P, nb, 2, 2, P], BF16, name=f"wtb{k}")
                eng = nc.sync if k == 0 else nc.scalar
                eng.dma_start(
                    out=wtb,
                    in_=wtv[:, b0 : b0 + nb, :].rearrange(
                        "p b (ih oh q) -> p b ih oh q", ih=2, oh=2, q=P
                    ),
                )
                wtbs.append((b0, wtb))
                b0 += nb

            def wslice(b, ih, oh):
                for bb0, wtb in reversed(wtbs):
                    if b >= bb0:
                        return wtb[:, b - bb0, ih, oh, :]
                raise AssertionError

            # ---- xtype broadcast via K=1 matmul; onehot vs fp32 iota ----
            xtypeB_ps = pso.tile([NUM_T, ROWS], F32, name="xtypeB_ps", space="PSUM")
            nc.tensor.matmul(
                out=xtypeB_ps[:],
                lhsT=ones[:],
                rhs=xtype_sb[:],
                start=True,
                stop=True,
            )

            iota_c = constp.tile([NUM_T, 1], F32, name="iota_c")
            nc.gpsimd.iota(
                iota_c[:],
                [[0, 1]],
                channel_multiplier=1,
                allow_small_or_imprecise_dtypes=True,
            )

            onehot = constp.tile([NUM_T, ROWS], BF16, name="onehot")
            nc.vector.tensor_tensor(
                out=onehot[:],
                in0=iota_c[:].to_broadcast([NUM_T, ROWS]),
                in1=xtypeB_ps[:],
                op=mybir.AluOpType.is_equal,
            )

            # ---- on-device column replication for the PE-broadcast bases ----
            wcb2 = constp.tile([NUM_T, PE_BASES, P], BF16, name="wcb2")
            nc.vector.tensor_copy(
                out=wcb2[:],
                in_=wcomp_sb[:, 0:PE_BASES]
                .rearrange("t (b one) -> t b one", one=1)
                .to_broadcast([NUM_T, PE_BASES, P]),
            )

            # ---- cb_all: one matmul computes w_comp[xtype[n], b] for all b ----
            cb_all_ps = pso.tile([NUM_B, ROWS], F32, name="cb_all_ps", space="PSUM")
            nc.tensor.matmul(
                out=cb_all_ps[:],
                lhsT=wcomp_sb[:],
                rhs=onehot[:],
                start=True,
                stop=True,
            )
            cb_all_sb = constp.tile([NUM_B, ROWS], BF16, name="cb_all_sb")
            nc.scalar.copy(cb_all_sb[:], cb_all_ps[:])
            # bounce to DRAM, then partition-broadcast reads (FIFO on sync q)
            nc.sync.dma_start(out=cbdram.ap()[:, :], in_=cb_all_sb[:])
            cbcats = []
            for g, (gb0, gb1) in enumerate(CB_GROUPS):
                nb = gb1 - gb0
                cbc = constp.tile([P, nb, ROWS], BF16, name=f"cbcat{g}")
                src = bass.AP(
                    tensor=cbdram.ap().tensor,
                    offset=gb0 * ROWS,
                    ap=[[0, P], [ROWS, nb], [1, ROWS]],
                )
                nc.sync.dma_start(out=cbc, in_=src)
                cbcats.append((gb0, cbc))

            def cb_rep(b):
                for gb0, cbc in reversed(cbcats):
                    if b >= gb0:
                        return cbc[:, b - gb0 : b - gb0 + 1, :].to_broadcast(
                            [P, 2, ROWS]
                        )
                raise AssertionError

            # ---- head bases via PE broadcast (hides bounce latency) ----
            head_cb = []
            for b in range(PE_BASES):
                cb_ps = psb.tile([P, ROWS], F32, name="cbps", tag="cbps", space="PSUM")
                nc.tensor.matmul(
                    out=cb_ps[:],
                    lhsT=wcb2[:, b, :],
                    rhs=onehot[:],
                    start=True,
                    stop=True,
                )
                cb_sb = cbp.tile([P, ROWS], BF16, name="cbsb", tag="cbsb")
                nc.scalar.copy(cb_sb[:], cb_ps[:])
                head_cb.append(cb_sb)

            # ---- per-basis scale + main matmuls ----
            psums = [
                pso.tile([P, ROWS], F32, name=f"out{oh}", space="PSUM")
                for oh in range(2)
            ]
            for b in range(NUM_B):
                if b < PE_BASES:
                    rep = head_cb[b][:].rearrange(
                        "p (one n) -> p one n", one=1
                    ).to_broadcast([P, 2, ROWS])
                else:
                    rep = cb_rep(b)
                u = up.tile([P, 2 * ROWS], BF16, name="u", tag="u")
                nc.vector.tensor_tensor(
                    out=u[:].rearrange("p (ih n) -> p ih n", ih=2),
                    in0=xtcat[:].rearrange("p (ih n) -> p ih n", ih=2),
                    in1=rep,
                    op=mybir.AluOpType.mult,
                )
                # last basis runs oh-major so psums[0] closes 2 matmuls
                # early and its drain overlaps the final matmuls
                pairs = (
                    [(ih, oh) for oh in range(2) for ih in range(2)]
                    if b == NUM_B - 1
                    else [(ih, oh) for ih in range(2) for oh in range(2)]
                )
                for ih, oh in pairs:
                    kt = b * 2 + ih
                    nc.tensor.matmul(
                        out=psums[oh][:],
                        lhsT=wslice(b, ih, oh),
                        rhs=u[:, ih * ROWS : (ih + 1) * ROWS],
                        start=(kt == 0),
                        stop=(kt == KT - 1),
                    )

            # ---- drain outT: oh0 via scalar, oh1 via vector (parallel) ----
            ot0 = outp.tile([P, ROWS], F32, name="ot0")
            nc.scalar.copy(ot0[:], psums[0][:])
            nc.sync.dma_start(out=outT.ap()[0:P, :], in_=ot0)
            ot1 = outp.tile([P, ROWS], F32, name="ot1")
            nc.vector.tensor_copy(out=ot1[:], in_=psums[1][:])
            nc.scalar.dma_start(out=outT.ap()[P : 2 * P, :], in_=ot1)

    nc.compile()
    return nc


_PROGRAM = None
LAST_RESULT = None  # test harness introspection


def kernel(x, xtype, weight, w_comp, trace=False):
    global _PROGRAM, LAST_RESULT
    x = np.asarray(x, dtype=np.float32)
    xtype = np.asarray(xtype)
    weight = np.asarray(weight, dtype=np.float32)
    w_comp = np.asarray(w_comp, dtype=np.float32)
    assert x.shape == (N_FULL, IN_DIM) and weight.shape == (NUM_B, IN_DIM * OUT_DIM)

    if _PROGRAM is None:
        _PROGRAM = _build_program()
    nc = _PROGRAM

    # type ids are 0..63: exact in bf16
    xtypeb_full = xtype.astype(np.float32).astype(NPBF16)
    wcomp_host = np.ascontiguousarray(w_comp).astype(NPBF16)
    # packed weight [p, b, ih, o]
    wt_host = np.ascontiguousarray(
        weight.reshape(NUM_B, 2, P, OUT_DIM).transpose(2, 0, 1, 3).reshape(P, -1)
    ).astype(NPBF16)
    in_maps = []
    for c in range(N_CORES):
        s = slice(c * ROWS, (c + 1) * ROWS)
        in_maps.append(
            {
                "xT": np.ascontiguousarray(
                    x[s].T.reshape(2, P, ROWS).transpose(1, 0, 2).reshape(P, 2 * ROWS)
                ).astype(NPBF16),
                "xtypeb": np.ascontiguousarray(xtypeb_full[s]).reshape(1, ROWS),
                "wcomp": wcomp_host,
                "wt": wt_host,
            }
        )

    res = run_bass_kernel_spmd(nc, in_maps, list(range(N_CORES)), trace=trace)
    LAST_RESULT = res

    out = np.empty((N_FULL, OUT_DIM), np.float32)
    for c in range(N_CORES):
        s = slice(c * ROWS, (c + 1) * ROWS)
        out[s] = res.results[c]["outT"].T
    return out


# revision 14
# speedup vs baseline: 1.0357x; 1.0357x over previous
"""Trainium2 Bass kernel for nn_DecompMultiTransform (RGCN basis-decomposition).

Reference computation:
    full_w = (w_comp @ weight).reshape(64, 256, 256)   # per-type weights
    out[n, :] = x[n, :] @ full_w[xtype[n]]             # N = 4096

Kernel formulation (avoids materializing the 16 MB full_w):
    onehot[t, n] = (xtype[n] == t)                     # [64, 512] per core
    cb_b[n]      = w_comp[xtype[n], b]  (via matmul with onehot)
    u_b[p, ihn]  = x^T * cb_b                          # scaled x halves
    outT[o, n]   = sum_{b,ih} weight-chunk^T @ u_b     # K=128 dense matmuls

Sharding: data-parallel over N across 8 cores (512 rows each); weight and
w_comp replicated. All math (type lookup, scaling, matmuls) runs on device;
host only does layout (transpose/pack) and dtype casts.

v3 changes vs v2 (51.9us):
  - PE warmup: dummy matmuls on memset tiles run during the framework
    preamble/input-DMA window so the tensor engine is at full p-state when
    real matmuls start (measured: first matmuls ran 760ns vs 454ns late).
  - cb mostly off the PE: one [16,512] matmul computes all 16 cb rows; they
    are partition-broadcast via a 16KB DRAM bounce + stride-0 broadcast
    reads (3 grouped DMAs). Only bases 0-1 keep the v2 PE-broadcast path to
    hide the bounce latency. PE: 80 -> 68 matmuls.
  - w_comp uploaded as 2KB [64,16] (was a 256KB column-replicated copy);
    the 2-base replication for the PE path is built on-device by the DVE.
  - output drain split across scalar + vector engines.
"""

import sys

if "/opt/trn_rl_repo" not in sys.path:
    sys.path.insert(0, "/opt/trn_rl_repo")

import numpy as np
import ml_dtypes

import concourse.bass as bass
import concourse.mybir as mybir
import concourse.tile as tile
from concourse import bacc
from concourse.bass_utils import run_bass_kernel_spmd

P = 128
N_FULL = 4096
IN_DIM = 256
OUT_DIM = 256
NUM_B = 16
NUM_T = 64
N_CORES = 8
ROWS = N_FULL // N_CORES          # 512 rows per core
KT = NUM_B * (IN_DIM // P)        # 32 contraction tiles of 128
W_BATCHES = (2, 2, 4, 8)          # weight DMA batch sizes (bases)
PE_BASES = 2                      # head bases with PE-broadcast cb
CB_GROUPS = ((2, 6), (6, 11), (11, 16))  # DRAM-bounce broadcast groups
N_DUMMY = 5                       # PE p-state warmup matmuls

F32 = mybir.dt.float32
BF16 = mybir.dt.bfloat16
I32 = mybir.dt.int32
NPBF16 = ml_dtypes.bfloat16


def _build_program():
    nc = bacc.Bacc("TRN2", target_bir_lowering=False, debug=False)

    xT = nc.declare_dram_parameter("xT", [P, 2 * ROWS], BF16, isOutput=False)
    xtype = nc.declare_dram_parameter("xtype", [ROWS], I32, isOutput=False)
    wcomp = nc.declare_dram_parameter("wcomp", [NUM_T, NUM_B], BF16, isOutput=False)
    # packed weight: wt[p, b, ih, o] = weight[b, (ih*128+p)*256 + o]
    wt = nc.declare_dram_parameter("wt", [P, NUM_B * 2 * OUT_DIM], BF16, isOutput=False)
    outT = nc.declare_dram_parameter("outT", [OUT_DIM, ROWS], F32, isOutput=True)

    cbdram = nc.dram_tensor("cb_bounce", [NUM_B, ROWS], BF16)
    wtv = wt.ap().rearrange("p (b r) -> p b r", b=NUM_B)

    with tile.TileContext(nc) as tc:
        with (
            tc.tile_pool(name="const", bufs=1) as constp,
            tc.tile_pool(name="wpool", bufs=1) as wpool,
            tc.tile_pool(name="cbp", bufs=2) as cbp,
            tc.tile_pool(name="up", bufs=4) as up,
            tc.tile_pool(name="outp", bufs=1) as outp,
            tc.tile_pool(name="psb", bufs=2, space="PSUM") as psb,
            tc.tile_pool(name="pso", bufs=1, space="PSUM") as pso,
        ):
            # ---- PE warmup: memset scratch, then dummy matmuls ----
            dlhs = constp.tile([P, P], BF16, name="dlhs")
            drhs = constp.tile([P, ROWS], BF16, name="drhs")
            nc.vector.memset(dlhs[:], 0)
            nc.vector.memset(drhs[:], 0)
            dps = pso.tile([P, ROWS], F32, name="dps", space="PSUM")
            for _ in range(N_DUMMY):
                nc.tensor.matmul(
                    out=dps[:], lhsT=dlhs[:], rhs=drhs[:], start=True, stop=True
                )

            # ---- sync-queue DMAs in priority order ----
            xtypeB = constp.tile([NUM_T, ROWS], I32, name="xtypeB")
            xtype_bcast = bass.AP(
                tensor=xtype.ap().tensor,
                offset=0,
                ap=[[0, NUM_T], [1, ROWS]],
            )
            nc.sync.dma_start(out=xtypeB[:], in_=xtype_bcast)

            wcomp_sb = constp.tile([NUM_T, NUM_B], BF16, name="wcomp_sb")
            nc.sync.dma_start(out=wcomp_sb[:], in_=wcomp.ap()[:, :])

            wtbs = []
            b0 = 0
            for k, nb in enumerate(W_BATCHES):
                wtb = wpool.tile([P, nb, 2, 2, P], BF16, name=f"wtb{k}")
                eng = nc.sync if k == 0 else nc.scalar
                eng.dma_start(
                    out=wtb,
                    in_=wtv[:, b0 : b0 + nb, :].rearrange(
                        "p b (ih oh q) -> p b ih oh q", ih=2, oh=2, q=P
                    ),
                )
                wtbs.append((b0, wtb))
                b0 += nb

            def wslice(b, ih, oh):
                for bb0, wtb in reversed(wtbs):
                    if b >= bb0:
                        return wtb[:, b - bb0, ih, oh, :]
                raise AssertionError

            xtcat = constp.tile([P, 2 * ROWS], BF16, name="xtcat")
            nc.sync.dma_start(out=xtcat, in_=xT.ap()[:, :])

            # ---- iota + onehot ----
            iota_c = constp.tile([NUM_T, 1], I32, name="iota_c")
            nc.gpsimd.iota(iota_c[:], [[0, 1]], channel_multiplier=1)

            onehot = constp.tile([NUM_T, ROWS], BF16, name="onehot")
            nc.vector.tensor_tensor(
                out=onehot[:],
                in0=iota_c[:].to_broadcast([NUM_T, ROWS]),
                in1=xtypeB[:],
                op=mybir.AluOpType.is_equal,
            )

            # ---- on-device column replication for the PE-broadcast bases ----
            wcb2 = constp.tile([NUM_T, PE_BASES, P], BF16, name="wcb2")
            nc.vector.tensor_copy(
                out=wcb2[:],
                in_=wcomp_sb[:, 0:PE_BASES]
                .rearrange("t (b one) -> t b one", one=1)
                .to_broadcast([NUM_T, PE_BASES, P]),
            )

            # ---- cb_all: one matmul computes w_comp[xtype[n], b] for all b ----
            cb_all_ps = pso.tile([NUM_B, ROWS], F32, name="cb_all_ps", space="PSUM")
            nc.tensor.matmul(
                out=cb_all_ps[:],
                lhsT=wcomp_sb[:],
                rhs=onehot[:],
                start=True,
                stop=True,
            )
            cb_all_sb = constp.tile([NUM_B, ROWS], BF16, name="cb_all_sb")
            nc.scalar.copy(cb_all_sb[:], cb_all_ps[:])
            # bounce to DRAM, then partition-broadcast reads (FIFO on sync q)
            nc.sync.dma_start(out=cbdram.ap()[:, :], in_=cb_all_sb[:])
            cbcats = []
            for g, (gb0, gb1) in enumerate(CB_GROUPS):
                nb = gb1 - gb0
                cbc = constp.tile([P, nb, ROWS], BF16, name=f"cbcat{g}")
                src = bass.AP(
                    tensor=cbdram.ap().tensor,
                    offset=gb0 * ROWS,
                    ap=[[0, P], [ROWS, nb], [1, ROWS]],
                )
                nc.sync.dma_start(out=cbc, in_=src)
                cbcats.append((gb0, cbc))

            def cb_rep(b):
                for gb0, cbc in reversed(cbcats):
                    if b >= gb0:
                        return cbc[:, b - gb0 : b - gb0 + 1, :].to_broadcast(
                            [P, 2, ROWS]
                        )
                raise AssertionError

            # ---- head bases via PE broadcast (hides bounce latency) ----
            head_cb = []
            for b in range(PE_BASES):
                cb_ps = psb.tile([P, ROWS], F32, name="cbps", tag="cbps", space="PSUM")
                nc.tensor.matmul(
                    out=cb_ps[:],
                    lhsT=wcb2[:, b, :],
                    rhs=onehot[:],
                    start=True,
                    stop=True,
                )
                cb_sb = cbp.tile([P, ROWS], BF16, name="cbsb", tag="cbsb")
                nc.scalar.copy(cb_sb[:], cb_ps[:])
                head_cb.append(cb_sb)

            # ---- per-basis scale + main matmuls ----
            psums = [
                pso.tile([P, ROWS], F32, name=f"out{oh}", space="PSUM")
                for oh in range(2)
            ]
            for b in range(NUM_B):
                if b < PE_BASES:
                    rep = head_cb[b][:].rearrange(
                        "p (one n) -> p one n", one=1
                    ).to_broadcast([P, 2, ROWS])
                else:
                    rep = cb_rep(b)
                u = up.tile([P, 2 * ROWS], BF16, name="u", tag="u")
                nc.vector.tensor_tensor(
                    out=u[:].rearrange("p (ih n) -> p ih n", ih=2),
                    in0=xtcat[:].rearrange("p (ih n) -> p ih n", ih=2),
                    in1=rep,
                    op=mybir.AluOpType.mult,
                )
                for ih in range(2):
                    kt = b * 2 + ih
                    for oh in range(2):
                        nc.tensor.matmul(
                            out=psums[oh][:],
                            lhsT=wslice(b, ih, oh),
                            rhs=u[:, ih * ROWS : (ih + 1) * ROWS],
                            start=(kt == 0),
                            stop=(kt == KT - 1),
                        )

            # ---- drain outT: oh0 via scalar, oh1 via vector (parallel) ----
            ot0 = outp.tile([P, ROWS], F32, name="ot0")
            nc.scalar.copy(ot0[:], psums[0][:])
            nc.sync.dma_start(out=outT.ap()[0:P, :], in_=ot0)
            ot1 = outp.tile([P, ROWS], F32, name="ot1")
            nc.vector.tensor_copy(out=ot1[:], in_=psums[1][:])
            nc.scalar.dma_start(out=outT.ap()[P : 2 * P, :], in_=ot1)

    nc.compile()
    return nc


_PROGRAM = None
LAST_RESULT = None  # test harness introspection


def kernel(x, xtype, weight, w_comp, trace=False):
    global _PROGRAM, LAST_RESULT
    x = np.asarray(x, dtype=np.float32)
    xtype = np.asarray(xtype)
    weight = np.asarray(weight, dtype=np.float32)
    w_comp = np.asarray(w_comp, dtype=np.float32)
    assert x.shape == (N_FULL, IN_DIM) and weight.shape == (NUM_B, IN_DIM * OUT_DIM)

    if _PROGRAM is None:
        _PROGRAM = _build_program()
    nc = _PROGRAM

    xtype32 = xtype.astype(np.int32)
    wcomp_host = np.ascontiguousarray(w_comp).astype(NPBF16)
    # packed weight [p, b, ih, o]
    wt_host = np.ascontiguousarray(
        weight.reshape(NUM_B, 2, P, OUT_DIM).transpose(2, 0, 1, 3).reshape(P, -1)
    ).astype(NPBF16)
    in_maps = []
    for c in range(N_CORES):
        s = slice(c * ROWS, (c + 1) * ROWS)
        in_maps.append(
            {
                "xT": np.ascontiguousarray(
                    x[s].T.reshape(2, P, ROWS).transpose(1, 0, 2).reshape(P, 2 * ROWS)
                ).astype(NPBF16),
                "xtype": np.ascontiguousarray(xtype32[s]),
                "wcomp": wcomp_host,
                "wt": wt_host,
            }
        )

    res = run_bass_kernel_spmd(nc, in_maps, list(range(N_CORES)), trace=trace)
    LAST_RESULT = res

    out = np.empty((N_FULL, OUT_DIM), np.float32)
    for c in range(N_CORES):
        s = slice(c * ROWS, (c + 1) * ROWS)
        out[s] = res.results[c]["outT"].T
    return out
